# revision 1
# baseline (speedup 1.0000x reference)
"""Head-parallel sparse-attention kernel for 8 Trainium2 NeuronCores.

Strategy (per sharding_hint): shard the H=8 heads (and the [H,N,N]
score/bias tensors) across the 8 devices; replicate x, batch, and the
small HxH / gate params; the head-interaction term (which mixes heads)
is realized as a psum_scatter across the head mesh axis; per-head
outputs are gathered before the wo projection.
"""
import numpy as np
import jax
import jax.numpy as jnp
from jax.sharding import Mesh, PartitionSpec as P
from jax.experimental.shard_map import shard_map
from functools import partial

H, N, DIN, DOUT = 8, 2048, 768, 1024
HD = DOUT // H
SCALE = 1.0 / float(np.sqrt(HD))
NEG = -1e9
EPS = 1e-5

_mesh = None
_jitted = None


def _get_mesh():
    global _mesh
    if _mesh is None:
        devs = jax.devices()[:8]
        _mesh = Mesh(np.array(devs), ('h',))
    return _mesh


def _head_block(q_blk, k_blk, v_blk, motif_blk, gqT_blk, gkT_blk, W_blk,
                batch, alpha, beta):
    # q_blk/k_blk/v_blk: [1, N, HD]; motif_blk: [1, N, N]
    # gqT_blk/gkT_blk: [1, N]; W_blk: [1, H] (my head's row of head_interaction)
    scores = jnp.einsum('hid,hjd->hij', q_blk, k_blk) * SCALE        # [1,N,N]
    mask = (batch[:, None] == batch[None, :])[None, :, :]
    scores = jnp.where(mask, scores, NEG)
    attn_base = alpha * scores + beta * motif_blk
    attn_final = scores + jnp.swapaxes(attn_base, 1, 2)
    attn_final = attn_final + scores * (gqT_blk[:, :, None] + gkT_blk[:, None, :])
    # head interaction: res[m,i,j] = sum_h scores[h,i,j] * W[h,m]
    contrib = scores[0][None, :, :] * W_blk[0][:, None, None]        # [H,N,N]
    res = jax.lax.psum_scatter(contrib, 'h', scatter_dimension=0, tiled=True)
    attn_final = attn_final + res                                    # [1,N,N]
    probs = jax.nn.softmax(attn_final, axis=-1)
    out_blk = jnp.einsum('hij,hjd->hid', probs, v_blk)               # [1,N,HD]
    return attn_final, out_blk


def _build():
    mesh = _get_mesh()
    sharded = shard_map(
        _head_block, mesh=mesh,
        in_specs=(P('h'), P('h'), P('h'), P('h'), P('h'), P('h'), P('h'),
                  P(), P(), P()),
        out_specs=(P('h'), P('h')),
    )

    @jax.jit
    def run(x, batch, motif, wq_w, wq_b, wk_w, wk_b, wv_w, wv_b,
            wo_w, wo_b, ln_g, ln_b, alpha, beta,
            gate_q_w, gate_q_b, gate_k_w, gate_k_b, head_interaction):
        q_flat = x @ wq_w + wq_b                                     # [N, DOUT]
        k_flat = x @ wk_w + wk_b
        v_flat = x @ wv_w + wv_b
        q = q_flat.reshape(N, H, HD).transpose(1, 0, 2)              # [H,N,HD]
        k = k_flat.reshape(N, H, HD).transpose(1, 0, 2)
        v = v_flat.reshape(N, H, HD).transpose(1, 0, 2)
        gqT = jnp.tanh(q_flat @ gate_q_w + gate_q_b).T               # [H,N]
        gkT = jnp.tanh(k_flat @ gate_k_w + gate_k_b).T
        attn_final, out = sharded(q, k, v, motif, gqT, gkT,
                                  head_interaction, batch, alpha, beta)
        out = out.transpose(1, 0, 2).reshape(N, DOUT)
        y = out @ wo_w + wo_b + x
        mu = jnp.mean(y, axis=-1, keepdims=True)
        var = jnp.mean(jnp.square(y - mu), axis=-1, keepdims=True)
        y = (y - mu) * jax.lax.rsqrt(var + EPS) * ln_g + ln_b
        return y, attn_final

    return run


def kernel(**inputs):
    global _jitted
    if _jitted is None:
        _jitted = _build()
    f32 = lambda a: jnp.asarray(np.asarray(a), jnp.float32)
    batch = jnp.asarray(np.asarray(inputs['batch']).astype(np.int32))
    y, attn_final = _jitted(
        f32(inputs['x']), batch, f32(inputs['motif_to_atom_attn']),
        f32(inputs['wq_w']), f32(inputs['wq_b']),
        f32(inputs['wk_w']), f32(inputs['wk_b']),
        f32(inputs['wv_w']), f32(inputs['wv_b']),
        f32(inputs['wo_w']), f32(inputs['wo_b']),
        f32(inputs['ln_g']), f32(inputs['ln_b']),
        f32(inputs['alpha']), f32(inputs['beta']),
        f32(inputs['gate_q_w']), f32(inputs['gate_q_b']),
        f32(inputs['gate_k_w']), f32(inputs['gate_k_b']),
        f32(inputs['head_interaction']))
    return np.asarray(y), np.asarray(attn_final)


# revision 2
# speedup vs baseline: 1.0384x; 1.0384x over previous
"""Head-parallel sparse-attention kernel for 8 Trainium2 NeuronCores.

Strategy (per sharding_hint): shard the H=8 heads (and the [H,N,N]
score/bias tensors) across the 8 devices; replicate x, batch, and the
small HxH / gate params; the head-interaction term (which mixes heads)
is realized as a psum_scatter across the head mesh axis; per-head
outputs are gathered before the wo projection.
"""
import numpy as np
import jax
import jax.numpy as jnp
from jax.sharding import Mesh, PartitionSpec as P
from jax.experimental.shard_map import shard_map
from functools import partial

H, N, DIN, DOUT = 8, 2048, 768, 1024
HD = DOUT // H
SCALE = 1.0 / float(np.sqrt(HD))
NEG = -1e9
EPS = 1e-5

_mesh = None
_jitted = None


def _get_mesh():
    global _mesh
    if _mesh is None:
        devs = jax.devices()[:8]
        _mesh = Mesh(np.array(devs), ('h',))
    return _mesh


def _head_block(q_blk, k_blk, v_blk, motif_blk, gqT_blk, gkT_blk, W_blk,
                batch, alpha, beta):
    # q_blk/k_blk/v_blk: [1, N, HD]; motif_blk: [1, N, N]
    # gqT_blk/gkT_blk: [1, N]; W_blk: [1, H] (my head's row of head_interaction)
    scores = jnp.einsum('hid,hjd->hij', q_blk, k_blk) * SCALE        # [1,N,N]
    mask = (batch[:, None] == batch[None, :])[None, :, :]
    scores = jnp.where(mask, scores, NEG)
    attn_base = alpha * scores + beta * motif_blk
    attn_final = scores + jnp.swapaxes(attn_base, 1, 2)
    attn_final = attn_final + scores * (gqT_blk[:, :, None] + gkT_blk[:, None, :])
    # head interaction: res[m,i,j] = sum_h scores[h,i,j] * W[h,m]
    contrib = scores[0][None, :, :] * W_blk[0][:, None, None]        # [H,N,N]
    res = jax.lax.psum_scatter(contrib, 'h', scatter_dimension=0, tiled=True)
    attn_final = attn_final + res                                    # [1,N,N]
    probs = jax.nn.softmax(attn_final, axis=-1)
    out_blk = jnp.einsum('hij,hjd->hid', probs, v_blk)               # [1,N,HD]
    return attn_final, out_blk


def _build():
    mesh = _get_mesh()
    sharded = shard_map(
        _head_block, mesh=mesh,
        in_specs=(P('h'), P('h'), P('h'), P('h'), P('h'), P('h'), P('h'),
                  P(), P(), P()),
        out_specs=(P('h'), P('h')),
    )

    from jax.sharding import NamedSharding
    rep = NamedSharding(mesh, P())
    shd = NamedSharding(mesh, P('h'))
    in_sh = (rep, rep, shd) + (rep,) * 17

    @partial(jax.jit, in_shardings=in_sh, out_shardings=(rep, shd))
    def run(x, batch, motif, wq_w, wq_b, wk_w, wk_b, wv_w, wv_b,
            wo_w, wo_b, ln_g, ln_b, alpha, beta,
            gate_q_w, gate_q_b, gate_k_w, gate_k_b, head_interaction):
        q_flat = x @ wq_w + wq_b                                     # [N, DOUT]
        k_flat = x @ wk_w + wk_b
        v_flat = x @ wv_w + wv_b
        q = q_flat.reshape(N, H, HD).transpose(1, 0, 2)              # [H,N,HD]
        k = k_flat.reshape(N, H, HD).transpose(1, 0, 2)
        v = v_flat.reshape(N, H, HD).transpose(1, 0, 2)
        gqT = jnp.tanh(q_flat @ gate_q_w + gate_q_b).T               # [H,N]
        gkT = jnp.tanh(k_flat @ gate_k_w + gate_k_b).T
        attn_final, out = sharded(q, k, v, motif, gqT, gkT,
                                  head_interaction, batch, alpha, beta)
        out = out.transpose(1, 0, 2).reshape(N, DOUT)
        y = out @ wo_w + wo_b + x
        mu = jnp.mean(y, axis=-1, keepdims=True)
        var = jnp.mean(jnp.square(y - mu), axis=-1, keepdims=True)
        y = (y - mu) * jax.lax.rsqrt(var + EPS) * ln_g + ln_b
        return y, attn_final

    return run


def kernel(**inputs):
    global _jitted
    if _jitted is None:
        _jitted = _build()
    f32 = lambda a: jnp.asarray(np.asarray(a), jnp.float32)
    batch = jnp.asarray(np.asarray(inputs['batch']).astype(np.int32))
    y, attn_final = _jitted(
        f32(inputs['x']), batch, f32(inputs['motif_to_atom_attn']),
        f32(inputs['wq_w']), f32(inputs['wq_b']),
        f32(inputs['wk_w']), f32(inputs['wk_b']),
        f32(inputs['wv_w']), f32(inputs['wv_b']),
        f32(inputs['wo_w']), f32(inputs['wo_b']),
        f32(inputs['ln_g']), f32(inputs['ln_b']),
        f32(inputs['alpha']), f32(inputs['beta']),
        f32(inputs['gate_q_w']), f32(inputs['gate_q_b']),
        f32(inputs['gate_k_w']), f32(inputs['gate_k_b']),
        f32(inputs['head_interaction']))
    return np.asarray(y), np.asarray(attn_final)
